# revision 8
# baseline (speedup 1.0000x reference)
"""Trainium2 Bass kernel for nn_AttentionModule (SAGAN-style 2D self-attention).

Per-sample computation (B=8 samples, one per NeuronCore, data-parallel):
    q = Wq @ x + bq         (32, 4096)
    k = Wk @ x + bk         (32, 4096)
    v = Wv @ x + bv         (256, 4096)
    attn = softmax(q^T k)   (4096, 4096), softmax over last dim
    y = v @ attn^T + x      (256, 4096)

Strategy per core:
  - Scores computed TRANSPOSED: Lt[m, n] = sum_d k[d,m] q[d,n], so the
    exp'd scores P land in SBUF with the contraction dim (m) on partitions,
    directly usable as the stationary operand of the AV matmul.
  - No softmax max-subtraction (|logits| < ~25, exp fits fp32 comfortably).
  - Softmax denominator comes free: the AV streaming operand v' carries an
    extra ones-column, so outT[n, 256] = colsum(P).
  - Logits matmuls (K = D = 32) are 4x row-packed via tile_position.
  - AV runs in bf16 (P and v' in bf16); logits and projections in f32r.
  - Final: per n-block normalize by 1/colsum, PE-transpose back to [c, n],
    add residual x, DMA out.

Host-side prep: weights are passed pre-transposed/replicated per the SBUF
layouts the kernel wants (free - kernel() receives full unsharded inputs).
"""

import numpy as np

import concourse.bacc as bacc
import concourse.bass as bass
import concourse.mybir as mybir
import concourse.tile as tile
from concourse.masks import make_identity

B, C, D = 8, 256, 32
HW = 4096                      # 64*64 pixels
NCH = 8                        # n-chunks of 512
CHUNK = 512
NB = 128                       # n-block
MB = 128                       # m-block
NMB = HW // MB                 # 32 m-blocks
F32 = mybir.dt.float32
F32R = mybir.dt.float32r
BF16 = mybir.dt.bfloat16
AF = mybir.ActivationFunctionType


def build_nc():
    nc = bacc.Bacc("TRN2", target_bir_lowering=False, debug=False)
    t = {}
    t["x"] = nc.dram_tensor("x", [C, HW], F32R, kind="ExternalInput").ap()
    t["wq4"] = nc.dram_tensor("wq4", [C, 128], F32R, kind="ExternalInput").ap()
    t["wk4"] = nc.dram_tensor("wk4", [C, 128], F32R, kind="ExternalInput").ap()
    t["bq4"] = nc.dram_tensor("bq4", [128, 1], F32, kind="ExternalInput").ap()
    t["bk4"] = nc.dram_tensor("bk4", [128, 1], F32, kind="ExternalInput").ap()
    t["wvtp"] = nc.dram_tensor("wvtp", [C, 258], F32R, kind="ExternalInput").ap()
    t["bvp"] = nc.dram_tensor("bvp", [1, 258], F32R, kind="ExternalInput").ap()
    t["ones1"] = nc.dram_tensor("ones1", [1, 128], F32R, kind="ExternalInput").ap()
    t["y"] = nc.dram_tensor("y", [C, HW], F32, kind="ExternalOutput").ap()

    with tile.TileContext(nc) as tc:
        _emit(nc, tc, t)
    nc.compile()
    return nc


def _emit(nc, tc, t):
    with (
        tc.tile_pool(name="const", bufs=1) as const,
        tc.tile_pool(name="sb", bufs=1) as sb,
        tc.tile_pool(name="stage", bufs=2) as stage,
    ):
        # ---- constants / weights -------------------------------------
        ident = const.tile([128, 128], F32)
        make_identity(nc, ident)
        ones1 = const.tile([1, 128], F32R)
        nc.sync.dma_start(ones1, t["ones1"])
        wq4 = const.tile([128, 2, 128], F32R)   # [c', cc, 4x32 q-weights]
        wk4 = const.tile([128, 2, 128], F32R)
        wvtp = const.tile([128, 2, 258], F32R)  # [c', cc, 256 v-w + 0-col]
        bq4 = const.tile([128, 1], F32)
        bk4 = const.tile([128, 1], F32)
        bvp = const.tile([1, 258], F32R)
        for cc in range(2):
            nc.sync.dma_start(wq4[:, cc, :], t["wq4"][128 * cc:128 * (cc + 1), :])
            nc.sync.dma_start(wk4[:, cc, :], t["wk4"][128 * cc:128 * (cc + 1), :])
            nc.sync.dma_start(wvtp[:, cc, :], t["wvtp"][128 * cc:128 * (cc + 1), :])
        nc.sync.dma_start(bq4, t["bq4"])
        nc.sync.dma_start(bk4, t["bk4"])
        nc.sync.dma_start(bvp, t["bvp"])

        # ---- persistent SBUF tensors ---------------------------------
        x0 = sb.tile([128, HW], F32R)         # x rows 0:128
        x1 = sb.tile([128, HW], F32R)         # x rows 128:256
        q4 = sb.tile([128, HW], F32R)         # q replicated 4x on partitions
        k4 = sb.tile([128, HW], F32R)
        vp = sb.tile([128, NMB * 258], BF16)   # v' tiles: [m-chunk 128, 257]
        pbuf = [sb.tile([128, 8 * 2048], BF16, tag=f"p{i}", name=f"p{i}")
                for i in range(2)]

        for ch in range(NCH):
            s = slice(CHUNK * ch, CHUNK * (ch + 1))
            nc.sync.dma_start(x0[:, s], t["x"][0:128, s])
            nc.sync.dma_start(x1[:, s], t["x"][128:256, s])
        xc = [x0, x1]

        # ---- phase 0: projections ------------------------------------
        with tc.tile_pool(name="ps0", bufs=2, space="PSUM") as ps0:
            # q4 / k4 = 4x-replicated q, k with bias via ACT copy
            for (w4, b4, dst) in ((wq4, bq4, q4), (wk4, bk4, k4)):
                for ch in range(NCH):
                    s = slice(CHUNK * ch, CHUNK * (ch + 1))
                    pt = ps0.tile([128, CHUNK], F32, tag="proj")
                    for cc in range(2):
                        nc.tensor.matmul(
                            pt, w4[:, cc, :], xc[cc][:, s],
                            start=(cc == 0), stop=(cc == 1),
                        )
                    nc.scalar.activation(dst[:, s], pt, AF.Identity, bias=b4)
            # v'T tiles (bf16), bias + ones-col via K=1 ones matmul
            for mb in range(NMB):
                ms = slice(MB * mb, MB * (mb + 1))
                vt = ps0.tile([128, 258], F32, tag="vt")
                for cc in range(2):
                    nc.tensor.matmul(
                        vt, xc[cc][:, ms], wvtp[:, cc, :],
                        start=(cc == 0), stop=False,
                    )
                nc.tensor.matmul(vt, ones1, bvp, start=False, stop=True)
                nc.vector.tensor_copy(vp[:, 258 * mb:258 * (mb + 1)], vt)

        # ---- main loop -----------------------------------------------
        with tc.tile_pool(name="ps1", bufs=1, space="PSUM") as ps1:

            def logits_group(ch, g):
                """4 row-packed matmuls (m-blocks 4g..4g+3) + exp to P."""
                lt = ps1.tile([128, 4 * CHUNK], F32, tag="lt", bufs=1)
                ns = slice(CHUNK * ch, CHUNK * (ch + 1))
                for r in range(4):
                    mb = 4 * g + r
                    nc.tensor.matmul(
                        lt[:, CHUNK * r:CHUNK * (r + 1)],
                        k4[32 * r:32 * (r + 1), MB * mb:MB * (mb + 1)],
                        q4[32 * r:32 * (r + 1), ns],
                        start=True, stop=True, tile_position=(32 * r, 0),
                    )
                dst = pbuf[ch % 2][:, 2048 * g:2048 * (g + 1)]
                nc.scalar.activation(dst, lt, AF.Exp)

            def av_block(ch, j, half):
                """16 accumulating AV matmuls (half of 32 m-chunks)."""
                p = pbuf[ch % 2]
                for mc in range(16 * half, 16 * (half + 1)):
                    g, i = divmod(mc, 4)
                    off = 2048 * g + CHUNK * i + NB * j
                    nc.tensor.matmul(
                        t["avps"], p[:, off:off + NB], vp[:, 258 * mc:258 * (mc + 1)],
                        start=(mc == 0), stop=(mc == 31),
                    )

            def finalize(ch, j, ysb):
                avps = t["avps"]
                recip = stage.tile([128, 1], F32, tag="recip")
                nc.vector.reciprocal(recip, avps[:, 256:257])
                normt = stage.tile([128, 256], F32, tag="normt")
                nc.vector.tensor_scalar_mul(normt, avps[:, 0:256], recip)
                trp = ps1.tile([128, 256], F32, tag="tr", bufs=2)
                for cb in range(2):
                    nc.tensor.transpose(
                        trp[:, 128 * cb:128 * (cb + 1)],
                        normt[:, 128 * cb:128 * (cb + 1)], ident)
                nb = 4 * ch + j
                for cb in range(2):
                    nc.vector.tensor_tensor(
                        out=ysb[cb][:, NB * j:NB * (j + 1)],
                        in0=trp[:, 128 * cb:128 * (cb + 1)],
                        in1=xc[cb][:, NB * nb:NB * (nb + 1)].bitcast(F32),
                        op=mybir.AluOpType.add,
                    )

            for ch in range(NCH + 1):
                ysb = None
                if ch > 0:
                    ysb = [stage.tile([128, CHUNK], F32, tag=f"y{cb}", name=f"ysb{cb}")
                           for cb in range(2)]
                for j in range(4):
                    if ch > 0:
                        t["avps"] = ps1.tile([128, 258], F32, tag="av", bufs=2,
                                             name="avps")
                    if ch < NCH:
                        logits_group(ch, 2 * j)
                    if ch > 0:
                        av_block(ch - 1, j, 0)
                    if ch < NCH:
                        logits_group(ch, 2 * j + 1)
                    if ch > 0:
                        av_block(ch - 1, j, 1)
                        finalize(ch - 1, j, ysb)
                if ch > 0:
                    s = slice(CHUNK * (ch - 1), CHUNK * ch)
                    nc.sync.dma_start(t["y"][0:128, s], ysb[0])
                    nc.sync.dma_start(t["y"][128:256, s], ysb[1])


# ---------------------------------------------------------------------
# host-side wrapper
# ---------------------------------------------------------------------
_CACHE = {}


def _prep_shared(Wq, bq, Wk, bk, Wv, bv):
    wq4 = np.tile(np.ascontiguousarray(Wq.T), (1, 4)).astype(np.float32)
    wk4 = np.tile(np.ascontiguousarray(Wk.T), (1, 4)).astype(np.float32)
    bq4 = np.tile(bq, 4).reshape(128, 1).astype(np.float32)
    bk4 = np.tile(bk, 4).reshape(128, 1).astype(np.float32)
    wvtp = np.concatenate(
        [Wv.T, np.zeros((C, 2), np.float32)], axis=1).astype(np.float32)
    bvp = np.concatenate([bv, [1.0, 0.0]]).reshape(1, 258).astype(np.float32)
    return {"wq4": np.ascontiguousarray(wq4), "wk4": np.ascontiguousarray(wk4),
            "bq4": bq4, "bk4": bk4,
            "wvtp": np.ascontiguousarray(wvtp), "bvp": bvp,
            "ones1": np.ones((1, 128), np.float32)}


def make_in_maps(x, Wq, bq, Wk, bk, Wv, bv):
    x = np.asarray(x, dtype=np.float32).reshape(B, C, HW)
    shared = _prep_shared(*(np.asarray(a, dtype=np.float32)
                            for a in (Wq, bq, Wk, bk, Wv, bv)))
    return [{"x": np.ascontiguousarray(x[b]), **shared} for b in range(B)]


def kernel(x, Wq, bq, Wk, bk, Wv, bv):
    from concourse.bass_utils import run_bass_kernel_spmd

    in_maps = make_in_maps(x, Wq, bq, Wk, bk, Wv, bv)
    if "nc" not in _CACHE:
        _CACHE["nc"] = build_nc()
    res = run_bass_kernel_spmd(_CACHE["nc"], in_maps, core_ids=list(range(B)))
    y = np.stack([res.results[b]["y"] for b in range(B)])
    return y.reshape(B, C, 64, 64).astype(np.float32)


# revision 10
# speedup vs baseline: 1.0239x; 1.0239x over previous
"""Trainium2 Bass kernel for nn_AttentionModule (SAGAN-style 2D self-attention).

Per-sample computation (B=8 samples, one per NeuronCore, data-parallel):
    q = Wq @ x + bq         (32, 4096)
    k = Wk @ x + bk         (32, 4096)
    v = Wv @ x + bv         (256, 4096)
    attn = softmax(q^T k)   (4096, 4096), softmax over last dim
    y = v @ attn^T + x      (256, 4096)

Strategy per core:
  - Scores computed TRANSPOSED: Lt[m, n] = sum_d k[d,m] q[d,n], so the
    exp'd scores P land in SBUF with the contraction dim (m) on partitions,
    directly usable as the stationary operand of the AV matmul.
  - No softmax max-subtraction (|logits| < ~25, exp fits fp32 comfortably).
  - Softmax denominator comes free: the AV streaming operand v' carries an
    extra ones-column, so outT[n, 256] = colsum(P).
  - Logits matmuls (K = D = 32) are 4x row-packed via tile_position.
  - AV runs in bf16 (P and v' in bf16); logits and projections in f32r.
  - Final: per n-block normalize by 1/colsum, PE-transpose back to [c, n],
    add residual x, DMA out.

Host-side prep: weights are passed pre-transposed/replicated per the SBUF
layouts the kernel wants (free - kernel() receives full unsharded inputs).
"""

import numpy as np

import concourse.bacc as bacc
import concourse.bass as bass
import concourse.mybir as mybir
import concourse.tile as tile
from concourse.masks import make_identity

B, C, D = 8, 256, 32
HW = 4096                      # 64*64 pixels
NCH = 8                        # n-chunks of 512
CHUNK = 512
NB = 128                       # n-block
MB = 128                       # m-block
NMB = HW // MB                 # 32 m-blocks
F32 = mybir.dt.float32
F32R = mybir.dt.float32r
BF16 = mybir.dt.bfloat16
FP16 = mybir.dt.float16
AF = mybir.ActivationFunctionType


def build_nc():
    nc = bacc.Bacc("TRN2", target_bir_lowering=False, debug=False)
    t = {}
    t["x"] = nc.dram_tensor("x", [C, HW], F32R, kind="ExternalInput").ap()
    t["wq4"] = nc.dram_tensor("wq4", [C, 128], F32R, kind="ExternalInput").ap()
    t["wk4"] = nc.dram_tensor("wk4", [C, 128], F32R, kind="ExternalInput").ap()
    t["bq4"] = nc.dram_tensor("bq4", [128, 1], F32, kind="ExternalInput").ap()
    t["bk4"] = nc.dram_tensor("bk4", [128, 1], F32, kind="ExternalInput").ap()
    t["wvtp"] = nc.dram_tensor("wvtp", [C, 258], F32R, kind="ExternalInput").ap()
    t["bvp"] = nc.dram_tensor("bvp", [1, 258], F32R, kind="ExternalInput").ap()
    t["ones1"] = nc.dram_tensor("ones1", [1, 128], F32R, kind="ExternalInput").ap()
    t["y"] = nc.dram_tensor("y", [C, HW], F32, kind="ExternalOutput").ap()

    with tile.TileContext(nc) as tc:
        _emit(nc, tc, t)
    nc.compile()
    return nc


def _emit(nc, tc, t):
    with (
        tc.tile_pool(name="const", bufs=1) as const,
        tc.tile_pool(name="sb", bufs=1) as sb,
        tc.tile_pool(name="stage", bufs=2) as stage,
    ):
        # ---- constants / weights -------------------------------------
        ident = const.tile([128, 128], F32)
        make_identity(nc, ident)
        ones1 = const.tile([1, 128], F32R)
        nc.sync.dma_start(ones1, t["ones1"])
        wq4 = const.tile([128, 2, 128], F32R)   # [c', cc, 4x32 q-weights]
        wk4 = const.tile([128, 2, 128], F32R)
        wvtp = const.tile([128, 2, 258], F32R)  # [c', cc, 256 v-w + 0-col]
        bq4 = const.tile([128, 1], F32)
        bk4 = const.tile([128, 1], F32)
        bvp = const.tile([1, 258], F32R)
        for cc in range(2):
            nc.sync.dma_start(wq4[:, cc, :], t["wq4"][128 * cc:128 * (cc + 1), :])
            nc.sync.dma_start(wk4[:, cc, :], t["wk4"][128 * cc:128 * (cc + 1), :])
            nc.sync.dma_start(wvtp[:, cc, :], t["wvtp"][128 * cc:128 * (cc + 1), :])
        nc.sync.dma_start(bq4, t["bq4"])
        nc.sync.dma_start(bk4, t["bk4"])
        nc.sync.dma_start(bvp, t["bvp"])

        # ---- persistent SBUF tensors ---------------------------------
        x0 = sb.tile([128, HW], F32R)         # x rows 0:128
        x1 = sb.tile([128, HW], F32R)         # x rows 128:256
        q4 = sb.tile([128, HW], FP16)         # q replicated 4x on partitions
        k4 = sb.tile([128, HW], FP16)
        vp = sb.tile([128, NMB * 258], BF16)   # v' tiles: [m-chunk 128, 257]
        pbuf = [sb.tile([128, 8 * 2048], BF16, tag=f"p{i}", name=f"p{i}")
                for i in range(2)]

        for ch in range(NCH):
            s = slice(CHUNK * ch, CHUNK * (ch + 1))
            nc.sync.dma_start(x0[:, s], t["x"][0:128, s])
            nc.sync.dma_start(x1[:, s], t["x"][128:256, s])
        xc = [x0, x1]

        # ---- phase 0: projections ------------------------------------
        with tc.tile_pool(name="ps0", bufs=2, space="PSUM") as ps0:
            # q4 / k4 = 4x-replicated q, k with bias via ACT copy
            for (w4, b4, dst) in ((wq4, bq4, q4), (wk4, bk4, k4)):
                for ch in range(NCH):
                    s = slice(CHUNK * ch, CHUNK * (ch + 1))
                    pt = ps0.tile([128, CHUNK], F32, tag="proj")
                    for cc in range(2):
                        nc.tensor.matmul(
                            pt, w4[:, cc, :], xc[cc][:, s],
                            start=(cc == 0), stop=(cc == 1),
                        )
                    nc.scalar.activation(dst[:, s], pt, AF.Identity, bias=b4)
            # v'T tiles (bf16), bias + ones-col via K=1 ones matmul
            for mb in range(NMB):
                ms = slice(MB * mb, MB * (mb + 1))
                vt = ps0.tile([128, 258], F32, tag="vt")
                for cc in range(2):
                    nc.tensor.matmul(
                        vt, xc[cc][:, ms], wvtp[:, cc, :],
                        start=(cc == 0), stop=False,
                    )
                nc.tensor.matmul(vt, ones1, bvp, start=False, stop=True)
                nc.vector.tensor_copy(vp[:, 258 * mb:258 * (mb + 1)], vt)

        # ---- main loop -----------------------------------------------
        with tc.tile_pool(name="ps1", bufs=1, space="PSUM") as ps1:

            def logits_group(ch, g):
                """4 row-packed matmuls (m-blocks 4g..4g+3) + exp to P."""
                lt = ps1.tile([128, 4 * CHUNK], F32, tag="lt", bufs=1)
                ns = slice(CHUNK * ch, CHUNK * (ch + 1))
                for r in range(4):
                    mb = 4 * g + r
                    nc.tensor.matmul(
                        lt[:, CHUNK * r:CHUNK * (r + 1)],
                        k4[32 * r:32 * (r + 1), MB * mb:MB * (mb + 1)],
                        q4[32 * r:32 * (r + 1), ns],
                        start=True, stop=True, tile_position=(32 * r, 0),
                    )
                dst = pbuf[ch % 2][:, 2048 * g:2048 * (g + 1)]
                nc.scalar.activation(dst, lt, AF.Exp)

            def av_block(ch, j, half):
                """16 accumulating AV matmuls (half of 32 m-chunks)."""
                p = pbuf[ch % 2]
                for mc in range(16 * half, 16 * (half + 1)):
                    g, i = divmod(mc, 4)
                    off = 2048 * g + CHUNK * i + NB * j
                    nc.tensor.matmul(
                        t["avps"], p[:, off:off + NB], vp[:, 258 * mc:258 * (mc + 1)],
                        start=(mc == 0), stop=(mc == 31),
                    )

            def finalize(ch, j, ysb):
                avps = t["avps"]
                recip = stage.tile([128, 1], F32, tag="recip")
                nc.vector.reciprocal(recip, avps[:, 256:257])
                normt = stage.tile([128, 256], F32, tag="normt")
                nc.vector.tensor_scalar_mul(normt, avps[:, 0:256], recip)
                trp = ps1.tile([128, 256], F32, tag="tr", bufs=2)
                for cb in range(2):
                    nc.tensor.transpose(
                        trp[:, 128 * cb:128 * (cb + 1)],
                        normt[:, 128 * cb:128 * (cb + 1)], ident)
                nb = 4 * ch + j
                for cb in range(2):
                    nc.vector.tensor_tensor(
                        out=ysb[cb][:, NB * j:NB * (j + 1)],
                        in0=trp[:, 128 * cb:128 * (cb + 1)],
                        in1=xc[cb][:, NB * nb:NB * (nb + 1)].bitcast(F32),
                        op=mybir.AluOpType.add,
                    )

            for ch in range(NCH + 1):
                ysb = None
                if ch > 0:
                    ysb = [stage.tile([128, CHUNK], F32, tag=f"y{cb}", name=f"ysb{cb}")
                           for cb in range(2)]
                for j in range(4):
                    if ch > 0:
                        t["avps"] = ps1.tile([128, 258], F32, tag="av", bufs=2,
                                             name="avps")
                    if ch < NCH:
                        logits_group(ch, 2 * j)
                    if ch > 0:
                        av_block(ch - 1, j, 0)
                    if ch < NCH:
                        logits_group(ch, 2 * j + 1)
                    if ch > 0:
                        av_block(ch - 1, j, 1)
                        finalize(ch - 1, j, ysb)
                if ch > 0:
                    s = slice(CHUNK * (ch - 1), CHUNK * ch)
                    nc.sync.dma_start(t["y"][0:128, s], ysb[0])
                    nc.sync.dma_start(t["y"][128:256, s], ysb[1])


# ---------------------------------------------------------------------
# host-side wrapper
# ---------------------------------------------------------------------
_CACHE = {}


def _prep_shared(Wq, bq, Wk, bk, Wv, bv):
    wq4 = np.tile(np.ascontiguousarray(Wq.T), (1, 4)).astype(np.float32)
    wk4 = np.tile(np.ascontiguousarray(Wk.T), (1, 4)).astype(np.float32)
    bq4 = np.tile(bq, 4).reshape(128, 1).astype(np.float32)
    bk4 = np.tile(bk, 4).reshape(128, 1).astype(np.float32)
    wvtp = np.concatenate(
        [Wv.T, np.zeros((C, 2), np.float32)], axis=1).astype(np.float32)
    bvp = np.concatenate([bv, [1.0, 0.0]]).reshape(1, 258).astype(np.float32)
    return {"wq4": np.ascontiguousarray(wq4), "wk4": np.ascontiguousarray(wk4),
            "bq4": bq4, "bk4": bk4,
            "wvtp": np.ascontiguousarray(wvtp), "bvp": bvp,
            "ones1": np.ones((1, 128), np.float32)}


def make_in_maps(x, Wq, bq, Wk, bk, Wv, bv):
    x = np.asarray(x, dtype=np.float32).reshape(B, C, HW)
    shared = _prep_shared(*(np.asarray(a, dtype=np.float32)
                            for a in (Wq, bq, Wk, bk, Wv, bv)))
    return [{"x": np.ascontiguousarray(x[b]), **shared} for b in range(B)]


def kernel(x, Wq, bq, Wk, bk, Wv, bv):
    from concourse.bass_utils import run_bass_kernel_spmd

    in_maps = make_in_maps(x, Wq, bq, Wk, bk, Wv, bv)
    if "nc" not in _CACHE:
        _CACHE["nc"] = build_nc()
    res = run_bass_kernel_spmd(_CACHE["nc"], in_maps, core_ids=list(range(B)))
    y = np.stack([res.results[b]["y"] for b in range(B)])
    return y.reshape(B, C, 64, 64).astype(np.float32)
